# revision 10
# baseline (speedup 1.0000x reference)
"""Trainium2 Bass kernel: batched time-domain cross-correlation.

Computes, for each of 2048 (=64x32) independent pairs (fp32):
    out[g, l] = sum_k d1[g, k + l - 301] * d2[g, k],   l in [0, 603)

Algorithm: overlap-save block correlation in a half-shift (negacyclic)
real-DFT basis; every matmul has a *shared* stationary operand (the
transform matrices) and batches all pairs in the moving operand:

  xp = d1 zero-padded/shifted; y = d2 zero-padded.
  out[B*c + j] = sum_v corr(w_{v+c}, y_v)[j]     (j in [0, B))
    w_s = xp-window at stride B, length N=2B
    y_v = y[B*v : B*v + B]    (blocks, zero-padded to N)
  Per-block circular corr via length-N negacyclic real DFT:
    bins k: Ur[k] = sum_n u[n] cos(pi n (2k+1)/N)
            Ui[k] = -sum_n u[n] sin(pi n (2k+1)/N),  k in [0, B)
    Z = X * conj(Y):  Zr = XrYr + XiYi ; Zi = XiYr - XrYi
    z[0:B] = Minv @ [Zr; Zi]  (exact: aliasing only corrupts j > B)

vs the first version of this kernel:
  - xp keeps only 45 leading / 27 trailing zeros (24 chunks, not 30);
    window chunks falling in the implicit zero region are skipped
    (x-forward matmul rows 82944 -> 72192 per core).
  - Zi is a single plane (d = XiYr - XrYi subtracted on DVE), so the
    inverse has 2 stationary planes, not 3 (rows 13824 -> 7680), and
    (c=1, jg=2) lag outputs >= 640 are never computed (lags end at 602).
  - product/tree ops are fused across all 6 bin groups per op (fewer,
    fatter DVE instructions; the fixed SBUF-access cost per op is large).
  - a tunable subset of the product muls runs on GpSimd.

Sharding: data-parallel over the 2048 pairs, 256 pairs per core, 8 cores.
"""

import math
import os
import sys

import ml_dtypes
import numpy as np

if "/opt/trn_rl_repo" not in sys.path:  # harness safety; axon site usually set
    sys.path.insert(0, "/opt/trn_rl_repo")

import concourse.bacc as bacc
import concourse.bass as bass
import concourse.mybir as mybir
import concourse.tile as tile
from concourse.bass_utils import run_bass_kernel_spmd

# ---- problem constants (hardcoded per contest contract) ----
NB_PAIRS, NCH, NT = 64, 32, 3000
LAGS = 603
SHIFT = 301  # NLAG + 1
NCORES = 8
G = (NB_PAIRS * NCH) // NCORES  # 256 pairs per core

# ---- algorithm constants ----
B = 384  # lag/block granularity; N = 2B
N = 2 * B
V = 8  # y blocks (ceil 3000/384)
C = 2  # output lag blocks (ceil 603/384)
S = V + C - 1  # x windows
BS = B // 128  # 3
NQ = N // 128  # 6 contraction chunks of a full window
NJ = B // 128  # 3
NR = 2 * NJ  # 6 bin groups (Re 0..2, Im 3..5)
XLEAD = 45  # leading zeros kept in xp (301 = 2*128 + 45)
U = 24  # xp chunks: 45 + 3000 + 27 = 3072
W = 24  # y chunks: 3000 + 72 = 3072

# x-forward matmul emission: (q, s0, s1, start, stop) per (r, chunk).
# Window s uses xp chunk u = 3s + q - 2; chunks u<0 / u>=24 are implicit
# zeros (skipped).  start/stop flags per psum column range:
#   w0 first=q2 last=q5; w1-6 first=q0 last=q5; w7 first=q0 last=q4;
#   w8 first=q0 last=q1.
XMM = [
    (0, 1, 9, True, False),
    (1, 1, 9, False, False),
    (2, 0, 8, False, False),
    (3, 0, 8, False, False),
    (4, 0, 8, False, False),
    (5, 0, 7, False, True),
]

# inverse output groups: (jg, cp) — lag l = 384c + 128jg + p; lags < 603
# need (c=0, jg=0..2) and (c=1, jg=0..1).
INVJG = [(0, 2), (1, 2), (2, 1)]

DT_MM = mybir.dt.bfloat16
NP_MM = ml_dtypes.bfloat16

# pairs per chunk: psum for x-fwd is [gh, S] fp32 <= 512 -> gh <= 56.
# Small head chunk (overlap DMA latency + PE ramp), geometric tail so the
# product drain after the last forwards is short.
_CH = [int(t) for t in os.environ.get(
    "KCHUNKS", "16,56,56,56,40,16,8,8").split(",")]
assert sum(_CH) == G
_chunks = []
_g = 0
for _w in _CH:
    _chunks.append((_g, _w))
    _g += _w
GHMAX = max(gh for _, gh in _chunks)
# inverse groups == chunks (inverse for chunk i is emitted during chunk i+1)
_IGROUPS = list(_chunks)

# which product muls go to GpSimd: list of (c, op) with op in
# {"p1", "p2a", "p2b"}; tuned so Pool busy ~= DVE busy ~= PE busy.
# KPOOL applies to steady-state chunks, KPOOLD to the drain chunks
# (the last DRAIN_CH chunks, where products must finish fast).
def _parse_pool(s):
    out = set()
    if s:
        for tok in s.split(","):
            c_, nm = tok.split(":")
            out.add((int(c_), nm))
    return out

POOL_SET = _parse_pool(os.environ.get("KPOOL", "0:p2a,1:p2a,1:p2b"))
POOL_SET_M = _parse_pool(
    os.environ.get("KPOOLM", "0:p2a,0:p2b,1:p2a,1:p2b"))
POOL_SET_D = _parse_pool(os.environ.get("KPOOLD", "0:p2a,1:p2b"))
DRAIN_CH = int(os.environ.get("KDRAIN", "3"))
MID_CH = set(
    int(t) for t in os.environ.get("KMID", "1,2,3").split(",") if t)

_PE_CACHE = {}
LAST_EXEC_NS = None
LAST_TRACE = None


def _matrices():
    n = np.arange(N, dtype=np.float64)[:, None]
    k = np.arange(B, dtype=np.float64)[None, :]
    theta = np.pi * n * (2 * k + 1) / N
    ffull = np.concatenate([np.cos(theta), -np.sin(theta)], axis=1)  # [N, 2B]
    minv = np.linalg.inv(ffull.T)[:B, :]  # [B, 2B]
    return ffull.astype(np.float32), minv.astype(np.float32)


def _const_tiles():
    """FW [128, NR*NQ*128]: FW[i, ((r*NQ)+q)*128 + col] = Ffull[128q+i, 128r+col]
    MT [128, 2*NJ*NJ*128]: for pl in {Mr, Mi}:
        MT[i, ((pl*NJ + rh)*NJ + jg)*128 + col] = M[128jg + col, 128rh + i]
    """
    ffull, minv = _matrices()
    fw = np.zeros((128, NR * NQ * 128), dtype=np.float32)
    for q in range(NQ):
        for r in range(NR):
            fw[:, (r * NQ + q) * 128 : (r * NQ + q + 1) * 128] = ffull[
                128 * q : 128 * (q + 1), 128 * r : 128 * (r + 1)
            ]
    mr = minv[:, :B]
    mi = minv[:, B:]
    mats = [mr, mi]
    mt = np.zeros((128, 2 * NJ * NJ * 128), dtype=np.float32)
    for pl in range(2):
        for rh in range(NJ):
            for jg in range(NJ):
                blk = mats[pl][128 * jg : 128 * (jg + 1), 128 * rh : 128 * (rh + 1)]
                base = ((pl * NJ + rh) * NJ + jg) * 128
                mt[:, base : base + 128] = blk.T
    return fw, mt


def build_kernel():
    nc = bacc.Bacc(
        "TRN2",
        target_bir_lowering=False,
        debug=False,
        num_devices=NCORES,
    )

    xp_d = nc.dram_tensor("xp", [128, G, U], DT_MM, kind="ExternalInput")
    yp_d = nc.dram_tensor("yp", [128, G, W], DT_MM, kind="ExternalInput")
    fw_d = nc.dram_tensor("fw", [128, NR * NQ * 128], DT_MM, kind="ExternalInput")
    mt_d = nc.dram_tensor("mt", [128, 2 * NJ * NJ * 128], DT_MM, kind="ExternalInput")
    out_d = nc.dram_tensor("out", [128, G, 5], mybir.dt.float32,
                           kind="ExternalOutput")

    with tile.TileContext(nc, trace_sim=False) as tc:
        with (
            tc.tile_pool(name="const", bufs=1) as cpool,
            tc.tile_pool(name="io", bufs=2) as iopool,
            tc.tile_pool(name="spec", bufs=2) as spool,
            tc.tile_pool(name="work", bufs=2) as wpool,
            tc.tile_pool(name="zpool", bufs=1) as zpool,
            tc.tile_pool(name="psum", bufs=1, space=bass.MemorySpace.PSUM) as ppool,
        ):
            fw_t = cpool.tile([128, NR * NQ * 128], DT_MM, tag="fw")
            mt_t = cpool.tile([128, 2 * NJ * NJ * 128], DT_MM, tag="mt")
            # spectra for all pairs: [128 bins-in-group, r-group, pair, slot]
            xs = cpool.tile([128, NR, G, S], DT_MM, tag="xs")
            ys = cpool.tile([128, NR, G, V], DT_MM, tag="ys")
            # accumulated spectral products, c-major so last free dim packs
            zr = zpool.tile([128, NJ, G, C], DT_MM, tag="zr")
            zi = zpool.tile([128, NJ, G, C], DT_MM, tag="zi")

            outt = iopool.tile([128, G, 5], mybir.dt.float32, tag="outt",
                               bufs=1)

            def emit_inverse(fgi):
                ig0, ign = _IGROUPS[fgi]
                gsl = slice(ig0, ig0 + ign)
                ps = ppool.tile([128, GHMAX, 5], mybir.dt.float32,
                                tag="psC", bufs=2)
                first = True
                slot = 0
                for jg, cp in INVJG:
                    psv = ps[:, :ign, slot : slot + cp]
                    nmm = 2 * NJ
                    i = 0
                    for pl, zt in ((0, zr), (1, zi)):
                        for rh in range(NJ):
                            # moving [ign, cp]: pair-major, c inner
                            rhs = zt[:, rh, gsl, 0:cp]
                            lhsT = mt_t[
                                :,
                                ((pl * NJ + rh) * NJ + jg) * 128 :
                                ((pl * NJ + rh) * NJ + jg + 1) * 128,
                            ]
                            nc.tensor.matmul(
                                psv, lhsT, rhs,
                                start=first, stop=(i == nmm - 1),
                            )
                            first = False
                            i += 1
                    slot += cp
                nc.scalar.copy(out=outt[:, gsl, :], in_=ps[:, :ign, :])
                nc.sync.dma_start(
                    out_d.ap()[:, gsl, :], outt[:, gsl, :]
                )

            inv_emitted = 0
            for ci, (g0, gh) in enumerate(_chunks):
                gsl = slice(g0, g0 + gh)
                xin = iopool.tile([128, GHMAX, U], DT_MM, tag="xin", bufs=3)
                yin = iopool.tile([128, GHMAX, W], DT_MM, tag="yin", bufs=3)
                if ci == 0:
                    nc.sync.dma_start(
                        fw_t[:, 0 : NQ * 128], fw_d.ap()[:, 0 : NQ * 128]
                    )
                nc.sync.dma_start(xin[:, :gh, :], xp_d.ap()[:, gsl, :])
                nc.sync.dma_start(yin[:, :gh, :], yp_d.ap()[:, gsl, :])
                if ci == 0:
                    for r in range(1, NR):
                        nc.sync.dma_start(
                            fw_t[:, r * NQ * 128 : (r + 1) * NQ * 128],
                            fw_d.ap()[:, r * NQ * 128 : (r + 1) * NQ * 128],
                        )
                    nc.sync.dma_start(mt_t[:], mt_d.ap())

                # ---- forward transforms, x and y interleaved per bin
                # group; small chunks pack several r-groups per psum bank so
                # one Act copy drains several (the per-op init cost is big)
                rpb = NR
                while rpb * gh * S > 512:
                    rpb -= 1
                rpb = max(1, min(rpb, 3))
                rb = 0
                while rb < NR:
                    nx = min(rpb, NR - rb)
                    ps = ppool.tile([128, nx * gh * S], mybir.dt.float32,
                                    tag="psA", bufs=3)
                    psx = ps[:, : nx * gh * S].rearrange(
                        "p (r g s) -> p r g s", r=nx, s=S
                    )
                    for ri in range(nx):
                        r = rb + ri
                        first = ri == 0
                        for q, s0, s1, st, sp in XMM:
                            lhsT = fw_t[
                                :, (r * NQ + q) * 128 : (r * NQ + q + 1) * 128
                            ]
                            u0 = 3 * s0 + q - 2
                            u1 = 3 * (s1 - 1) + q - 2
                            rhs = xin[:, 0:gh, u0 : u1 + 1 : 3]
                            nc.tensor.matmul(
                                psx[:, ri, :, s0:s1], lhsT, rhs,
                                start=(first and st),
                                stop=(sp and ri == nx - 1),
                            )
                    nc.scalar.copy(
                        out=xs[:, rb : rb + nx, gsl, :], in_=psx[:]
                    )
                    ny = nx
                    ps = ppool.tile([128, ny * gh * V], mybir.dt.float32,
                                    tag="psB", bufs=3)
                    psy = ps[:, : ny * gh * V].rearrange(
                        "p (r g v) -> p r g v", r=ny, v=V
                    )
                    for ri in range(ny):
                        r = rb + ri
                        for q in range(NJ):
                            lhsT = fw_t[
                                :, (r * NQ + q) * 128 : (r * NQ + q + 1) * 128
                            ]
                            rhs = yin[:, 0:gh, q : q + 3 * (V - 1) + 1 : 3]
                            nc.tensor.matmul(
                                psy[:, ri, :, :], lhsT, rhs,
                                start=(ri == 0 and q == 0),
                                stop=(ri == ny - 1 and q == NJ - 1),
                            )
                    nc.scalar.copy(
                        out=ys[:, rb : rb + ny, gsl, :], in_=psy[:]
                    )
                    rb += nx

                # deferred inverse AFTER this chunk's forward matmuls so the
                # PE queue never stalls on the product engines
                while (
                    inv_emitted < len(_IGROUPS)
                    and _IGROUPS[inv_emitted][0] + _IGROUPS[inv_emitted][1] <= g0
                ):
                    emit_inverse(inv_emitted)
                    inv_emitted += 1

                # ---- pointwise products + v-sum trees (DVE + GpSimd) ----
                if ci >= len(_chunks) - DRAIN_CH:
                    pset = POOL_SET_D
                elif ci in MID_CH:
                    pset = POOL_SET_M
                else:
                    pset = POOL_SET

                def eng(c_, nm):
                    return nc.gpsimd if (c_, nm) in pset else nc.vector

                with nc.allow_low_precision("bf16 spectra products"):
                    # Zr path: p1 dim0 order (half, c, rh); muls per
                    # (half, c), trees fused across everything
                    p1 = wpool.tile([128, 2 * C * NJ, GHMAX, V], DT_MM,
                                    tag="p1", bufs=2)
                    for h in range(2):
                        for c in range(C):
                            eng(c, "p1a" if h == 0 else "p1b").tensor_mul(
                                p1[:, (h * C + c) * NJ : (h * C + c + 1) * NJ,
                                   :gh, :],
                                xs[:, h * NJ : (h + 1) * NJ, gsl, c : c + V],
                                ys[:, h * NJ : (h + 1) * NJ, gsl, :],
                            )
                    t4 = wpool.tile([128, 2 * C * NJ, GHMAX, V // 2], DT_MM,
                                    tag="t4", bufs=2)
                    nc.vector.tensor_add(
                        t4[:, :, :gh, :], p1[:, :, :gh, 0 : V // 2],
                        p1[:, :, :gh, V // 2 : V],
                    )
                    t2 = wpool.tile([128, 2 * C * NJ, GHMAX, V // 4], DT_MM,
                                    tag="t2", bufs=2)
                    nc.vector.tensor_add(
                        t2[:, :, :gh, :], t4[:, :, :gh, 0 : V // 4],
                        t4[:, :, :gh, V // 4 : V // 2],
                    )
                    ta = wpool.tile([128, C * NJ, GHMAX, 2], DT_MM, tag="ta",
                                    bufs=2)
                    nc.vector.tensor_add(
                        ta[:, :, :gh, :], t2[:, 0 : C * NJ, :gh, :],
                        t2[:, C * NJ : 2 * C * NJ, :gh, :],
                    )
                    nc.vector.tensor_add(
                        zr[:, :, gsl, :].rearrange("p r g c -> p c r g"),
                        ta[:, :, :gh, 0].rearrange("p (c r) g -> p c r g", c=C),
                        ta[:, :, :gh, 1].rearrange("p (c r) g -> p c r g", c=C),
                    )
                    # Zi path: d = XiYr - XrYi summed over v; dim0 = (c, rh)
                    p2a = wpool.tile([128, C * NJ, GHMAX, V], DT_MM, tag="p2a",
                                     bufs=2)
                    p2b = wpool.tile([128, C * NJ, GHMAX, V], DT_MM, tag="p2b",
                                     bufs=2)
                    for c in range(C):
                        eng(c, "p2a").tensor_mul(
                            p2a[:, c * NJ : (c + 1) * NJ, :gh, :],
                            xs[:, NJ:NR, gsl, c : c + V],
                            ys[:, 0:NJ, gsl, :],
                        )
                        eng(c, "p2b").tensor_mul(
                            p2b[:, c * NJ : (c + 1) * NJ, :gh, :],
                            xs[:, 0:NJ, gsl, c : c + V],
                            ys[:, NJ:NR, gsl, :],
                        )
                    dd = wpool.tile([128, C * NJ, GHMAX, V], DT_MM, tag="dd",
                                    bufs=2)
                    nc.vector.tensor_sub(
                        dd[:, :, :gh, :], p2a[:, :, :gh, :], p2b[:, :, :gh, :]
                    )
                    u4 = wpool.tile([128, C * NJ, GHMAX, V // 2], DT_MM,
                                    tag="u4", bufs=2)
                    nc.vector.tensor_add(
                        u4[:, :, :gh, :], dd[:, :, :gh, 0 : V // 2],
                        dd[:, :, :gh, V // 2 : V],
                    )
                    u2 = wpool.tile([128, C * NJ, GHMAX, V // 4], DT_MM,
                                    tag="u2", bufs=2)
                    nc.vector.tensor_add(
                        u2[:, :, :gh, :], u4[:, :, :gh, 0 : V // 4],
                        u4[:, :, :gh, V // 4 : V // 2],
                    )
                    nc.vector.tensor_add(
                        zi[:, :, gsl, :].rearrange("p r g c -> p c r g"),
                        u2[:, :, :gh, 0].rearrange("p (c r) g -> p c r g", c=C),
                        u2[:, :, :gh, 1].rearrange("p (c r) g -> p c r g", c=C),
                    )

            while inv_emitted < len(_IGROUPS):
                emit_inverse(inv_emitted)
                inv_emitted += 1

    nc.compile()
    return nc


def _prep_core_inputs(d1f, d2f, fw, mt, core):
    """d1f/d2f: [2048, 3000] fp32. Returns the in_map for `core`."""
    sl = slice(core * G, (core + 1) * G)
    x = d1f[sl]
    y = d2f[sl]
    xp = np.zeros((G, U * 128), dtype=np.float32)
    xp[:, XLEAD : XLEAD + NT] = x
    yp = np.zeros((G, W * 128), dtype=np.float32)
    yp[:, :NT] = y
    # device layouts: xpT[p, g, u] = xp[g, 128u + p]
    xpT = np.ascontiguousarray(xp.reshape(G, U, 128).transpose(2, 0, 1)).astype(NP_MM)
    ypT = np.ascontiguousarray(yp.reshape(G, W, 128).transpose(2, 0, 1)).astype(NP_MM)
    return {"xp": xpT, "yp": ypT, "fw": fw.astype(NP_MM), "mt": mt.astype(NP_MM)}


def kernel(data1: np.ndarray, data2: np.ndarray) -> np.ndarray:
    import time

    d1f = np.ascontiguousarray(data1, dtype=np.float32).reshape(-1, NT)
    d2f = np.ascontiguousarray(data2, dtype=np.float32).reshape(-1, NT)
    fw, mt = _const_tiles()

    t0 = time.time()
    if "nc" not in _PE_CACHE:
        _PE_CACHE["nc"] = build_kernel()
    nc = _PE_CACHE["nc"]
    print(f"[kernel] build+compile {time.time() - t0:.1f}s", file=sys.stderr,
          flush=True)

    in_maps = [_prep_core_inputs(d1f, d2f, fw, mt, i) for i in range(NCORES)]
    t0 = time.time()
    res = run_bass_kernel_spmd(nc, in_maps, core_ids=list(range(NCORES)))
    print(f"[kernel] spmd run {time.time() - t0:.1f}s", file=sys.stderr, flush=True)
    global LAST_EXEC_NS, LAST_TRACE
    LAST_EXEC_NS = res.exec_time_ns
    LAST_TRACE = res.instructions_and_trace
    if res.exec_time_ns is not None:
        print(f"[kernel] HW exec {res.exec_time_ns} ns", file=sys.stderr, flush=True)

    # slot order: (jg0,c0) (jg0,c1) (jg1,c0) (jg1,c1) (jg2,c0)
    slot_of = {(0, 0): 0, (0, 1): 1, (1, 0): 2, (1, 1): 3, (2, 0): 4}
    outs = []
    for i in range(NCORES):
        o = res.results[i]["out"]  # [128, G, 5]
        full = np.empty((G, LAGS), dtype=np.float32)
        for jg in range(NJ):
            for c in range(C):
                if (jg, c) not in slot_of:
                    continue
                lo = 384 * c + 128 * jg
                if lo >= LAGS:
                    continue
                n = min(128, LAGS - lo)
                full[:, lo : lo + n] = o[:n, :, slot_of[(jg, c)]].T
        outs.append(full)
    return np.concatenate(outs, axis=0).reshape(NB_PAIRS, NCH, LAGS)


# revision 11
# speedup vs baseline: 1.1635x; 1.1635x over previous
"""Trainium2 Bass kernel: batched time-domain cross-correlation.

Computes, for each of 2048 (=64x32) independent pairs (fp32):
    out[g, l] = sum_k d1[g, k + l - 301] * d2[g, k],   l in [0, 603)

Algorithm: overlap-save block correlation in a half-shift (negacyclic)
real-DFT basis; every matmul has a *shared* stationary operand (the
transform matrices) and batches all pairs in the moving operand:

  xp = d1 zero-padded/shifted; y = d2 zero-padded.
  out[B*c + j] = sum_v corr(w_{v+c}, y_v)[j]     (j in [0, B))
    w_s = xp-window at stride B, length N=2B
    y_v = y[B*v : B*v + B]    (blocks, zero-padded to N)
  Per-block circular corr via length-N negacyclic real DFT:
    bins k: Ur[k] = sum_n u[n] cos(pi n (2k+1)/N)
            Ui[k] = -sum_n u[n] sin(pi n (2k+1)/N),  k in [0, B)
    Z = X * conj(Y):  Zr = XrYr + XiYi ; Zi = XiYr - XrYi
    z[0:B] = Minv @ [Zr; Zi]  (exact: aliasing only corrupts j > B)

vs the first version of this kernel:
  - xp keeps only 45 leading / 27 trailing zeros (24 chunks, not 30);
    window chunks falling in the implicit zero region are skipped
    (x-forward matmul rows 82944 -> 72192 per core).
  - Zi is a single plane (d = XiYr - XrYi subtracted on DVE), so the
    inverse has 2 stationary planes, not 3 (rows 13824 -> 7680), and
    (c=1, jg=2) lag outputs >= 640 are never computed (lags end at 602).
  - product/tree ops are fused across all 6 bin groups per op (fewer,
    fatter DVE instructions; the fixed SBUF-access cost per op is large).
  - a tunable subset of the product muls runs on GpSimd.

Sharding: data-parallel over the 2048 pairs, 256 pairs per core, 8 cores.
"""

import math
import os
import sys

import ml_dtypes
import numpy as np

if "/opt/trn_rl_repo" not in sys.path:  # harness safety; axon site usually set
    sys.path.insert(0, "/opt/trn_rl_repo")

import concourse.bacc as bacc
import concourse.bass as bass
import concourse.mybir as mybir
import concourse.tile as tile
from concourse.bass_utils import run_bass_kernel_spmd

# ---- problem constants (hardcoded per contest contract) ----
NB_PAIRS, NCH, NT = 64, 32, 3000
LAGS = 603
SHIFT = 301  # NLAG + 1
NCORES = 8
G = (NB_PAIRS * NCH) // NCORES  # 256 pairs per core

# ---- algorithm constants ----
B = 384  # lag/block granularity; N = 2B
N = 2 * B
V = 8  # y blocks (ceil 3000/384)
C = 2  # output lag blocks (ceil 603/384)
S = V + C - 1  # x windows
BS = B // 128  # 3
NQ = N // 128  # 6 contraction chunks of a full window
NJ = B // 128  # 3
NR = 2 * NJ  # 6 bin groups (Re 0..2, Im 3..5)
XLEAD = 45  # leading zeros kept in xp (301 = 2*128 + 45)
U = 24  # xp chunks: 45 + 3000 + 27 = 3072
W = 24  # y chunks: 3000 + 72 = 3072

# x-forward matmul emission: (q, s0, s1, start, stop) per (r, chunk).
# Window s uses xp chunk u = 3s + q - 2; chunks u<0 / u>=24 are implicit
# zeros (skipped).  start/stop flags per psum column range:
#   w0 first=q2 last=q5; w1-6 first=q0 last=q5; w7 first=q0 last=q4;
#   w8 first=q0 last=q1.
XMM = [
    (0, 1, 9, True, False),
    (1, 1, 9, False, False),
    (2, 0, 8, False, False),
    (3, 0, 8, False, False),
    (4, 0, 8, False, False),
    (5, 0, 7, False, True),
]

# inverse output groups: (jg, cp) — lag l = 384c + 128jg + p; lags < 603
# need (c=0, jg=0..2) and (c=1, jg=0..1).
INVJG = [(0, 2), (1, 2), (2, 1)]

DT_MM = mybir.dt.bfloat16
NP_MM = ml_dtypes.bfloat16

# pairs per chunk: psum for x-fwd is [gh, S] fp32 <= 512 -> gh <= 56.
# Small head chunk (overlap DMA latency + PE ramp), geometric tail so the
# product drain after the last forwards is short.
_CH = [int(t) for t in os.environ.get(
    "KCHUNKS", "16,56,56,56,40,16,8,8").split(",")]
assert sum(_CH) == G
_chunks = []
_g = 0
for _w in _CH:
    _chunks.append((_g, _w))
    _g += _w
GHMAX = max(gh for _, gh in _chunks)
# inverse groups == chunks (inverse for chunk i is emitted during chunk i+1)
_IGROUPS = list(_chunks)

# which product muls go to GpSimd: list of (c, op) with op in
# {"p1", "p2a", "p2b"}; tuned so Pool busy ~= DVE busy ~= PE busy.
# KPOOL applies to steady-state chunks, KPOOLD to the drain chunks
# (the last DRAIN_CH chunks, where products must finish fast).
def _parse_pool(s):
    out = set()
    if s:
        for tok in s.split(","):
            c_, nm = tok.split(":")
            out.add((int(c_), nm))
    return out

POOL_SET = _parse_pool(os.environ.get("KPOOL", "0:p2a,1:p2a,1:p2b"))
POOL_SET_M = _parse_pool(
    os.environ.get("KPOOLM", "0:p2a,0:p2b,1:p2a,1:p2b"))
POOL_SET_D = _parse_pool(os.environ.get("KPOOLD", "0:p2a,1:p2b"))
DRAIN_CH = int(os.environ.get("KDRAIN", "3"))
MID_CH = set(
    int(t) for t in os.environ.get("KMID", "1,2,3").split(",") if t)

_PE_CACHE = {}
LAST_EXEC_NS = None
LAST_TRACE = None


def _matrices():
    n = np.arange(N, dtype=np.float64)[:, None]
    k = np.arange(B, dtype=np.float64)[None, :]
    theta = np.pi * n * (2 * k + 1) / N
    ffull = np.concatenate([np.cos(theta), -np.sin(theta)], axis=1)  # [N, 2B]
    minv = np.linalg.inv(ffull.T)[:B, :]  # [B, 2B]
    return ffull.astype(np.float32), minv.astype(np.float32)


def _const_tiles():
    """FW [128, NR*NQ*128]: FW[i, ((r*NQ)+q)*128 + col] = Ffull[128q+i, 128r+col]
    MT [128, 2*NJ*NJ*128]: for pl in {Mr, Mi}:
        MT[i, ((pl*NJ + rh)*NJ + jg)*128 + col] = M[128jg + col, 128rh + i]
    """
    ffull, minv = _matrices()
    fw = np.zeros((128, NR * NQ * 128), dtype=np.float32)
    for q in range(NQ):
        for r in range(NR):
            fw[:, (r * NQ + q) * 128 : (r * NQ + q + 1) * 128] = ffull[
                128 * q : 128 * (q + 1), 128 * r : 128 * (r + 1)
            ]
    mr = minv[:, :B]
    mi = minv[:, B:]
    mats = [mr, mi]
    mt = np.zeros((128, 2 * NJ * NJ * 128), dtype=np.float32)
    for pl in range(2):
        for rh in range(NJ):
            for jg in range(NJ):
                blk = mats[pl][128 * jg : 128 * (jg + 1), 128 * rh : 128 * (rh + 1)]
                base = ((pl * NJ + rh) * NJ + jg) * 128
                mt[:, base : base + 128] = blk.T
    return fw, mt


def build_kernel():
    nc = bacc.Bacc(
        "TRN2",
        target_bir_lowering=False,
        debug=False,
        num_devices=NCORES,
    )

    xp_d = nc.dram_tensor("xp", [128, G, U], DT_MM, kind="ExternalInput")
    yp_d = nc.dram_tensor("yp", [128, G, W], DT_MM, kind="ExternalInput")
    fw_d = nc.dram_tensor("fw", [128, NR * NQ * 128], DT_MM, kind="ExternalInput")
    mt_d = nc.dram_tensor("mt", [128, 2 * NJ * NJ * 128], DT_MM, kind="ExternalInput")
    out_d = nc.dram_tensor("out", [128, G, 5], mybir.dt.float32,
                           kind="ExternalOutput")

    with tile.TileContext(nc, trace_sim=False) as tc:
        with (
            tc.tile_pool(name="const", bufs=1) as cpool,
            tc.tile_pool(name="io", bufs=2) as iopool,
            tc.tile_pool(name="spec", bufs=2) as spool,
            tc.tile_pool(name="work", bufs=2) as wpool,
            tc.tile_pool(name="zpool", bufs=1) as zpool,
            tc.tile_pool(name="psum", bufs=1, space=bass.MemorySpace.PSUM) as ppool,
        ):
            fw_t = cpool.tile([128, NR * NQ * 128], DT_MM, tag="fw")
            mt_t = cpool.tile([128, 2 * NJ * NJ * 128], DT_MM, tag="mt")
            # spectra for all pairs: [128 bins-in-group, r-group, pair, slot]
            xs = cpool.tile([128, NR, G, S], DT_MM, tag="xs")
            ys = cpool.tile([128, NR, G, V], DT_MM, tag="ys")
            # accumulated spectral products, c-major so last free dim packs
            zr = zpool.tile([128, NJ, G, C], DT_MM, tag="zr")
            zi = zpool.tile([128, NJ, G, C], DT_MM, tag="zi")

            outt = iopool.tile([128, G, 5], mybir.dt.float32, tag="outt",
                               bufs=1)

            def emit_inverse(fgi):
                ig0, ign = _IGROUPS[fgi]
                gsl = slice(ig0, ig0 + ign)
                ps = ppool.tile([128, GHMAX, 5], mybir.dt.float32,
                                tag="psC", bufs=2)
                first = True
                slot = 0
                for jg, cp in INVJG:
                    psv = ps[:, :ign, slot : slot + cp]
                    nmm = 2 * NJ
                    i = 0
                    for pl, zt in ((0, zr), (1, zi)):
                        for rh in range(NJ):
                            # moving [ign, cp]: pair-major, c inner
                            rhs = zt[:, rh, gsl, 0:cp]
                            lhsT = mt_t[
                                :,
                                ((pl * NJ + rh) * NJ + jg) * 128 :
                                ((pl * NJ + rh) * NJ + jg + 1) * 128,
                            ]
                            nc.tensor.matmul(
                                psv, lhsT, rhs,
                                start=first, stop=(i == nmm - 1),
                            )
                            first = False
                            i += 1
                    slot += cp
                nc.scalar.copy(out=outt[:, gsl, :], in_=ps[:, :ign, :])
                nc.sync.dma_start(
                    out_d.ap()[:, gsl, :], outt[:, gsl, :]
                )

            inv_emitted = 0
            for ci, (g0, gh) in enumerate(_chunks):
                gsl = slice(g0, g0 + gh)
                xin = iopool.tile([128, GHMAX, U], DT_MM, tag="xin", bufs=3)
                yin = iopool.tile([128, GHMAX, W], DT_MM, tag="yin", bufs=3)
                if ci == 0:
                    nc.sync.dma_start(
                        fw_t[:, 0 : NQ * 128], fw_d.ap()[:, 0 : NQ * 128]
                    )
                nc.sync.dma_start(xin[:, :gh, :], xp_d.ap()[:, gsl, :])
                nc.sync.dma_start(yin[:, :gh, :], yp_d.ap()[:, gsl, :])
                if ci == 0:
                    for r in range(1, NR):
                        nc.sync.dma_start(
                            fw_t[:, r * NQ * 128 : (r + 1) * NQ * 128],
                            fw_d.ap()[:, r * NQ * 128 : (r + 1) * NQ * 128],
                        )
                    nc.sync.dma_start(mt_t[:], mt_d.ap())

                # ---- forward transforms, x and y interleaved per bin
                # group; small chunks pack several r-groups per psum bank so
                # one Act copy drains several (the per-op init cost is big)
                rpb = NR
                while rpb * gh * S > 512:
                    rpb -= 1
                rpb = max(1, min(rpb, 3))
                rb = 0
                while rb < NR:
                    nx = min(rpb, NR - rb)
                    ps = ppool.tile([128, nx * gh * S], mybir.dt.float32,
                                    tag="psA", bufs=3)
                    psx = ps[:, : nx * gh * S].rearrange(
                        "p (r g s) -> p r g s", r=nx, s=S
                    )
                    for ri in range(nx):
                        r = rb + ri
                        first = ri == 0
                        for q, s0, s1, st, sp in XMM:
                            lhsT = fw_t[
                                :, (r * NQ + q) * 128 : (r * NQ + q + 1) * 128
                            ]
                            u0 = 3 * s0 + q - 2
                            u1 = 3 * (s1 - 1) + q - 2
                            rhs = xin[:, 0:gh, u0 : u1 + 1 : 3]
                            nc.tensor.matmul(
                                psx[:, ri, :, s0:s1], lhsT, rhs,
                                start=(first and st),
                                stop=(sp and ri == nx - 1),
                            )
                    nc.scalar.copy(
                        out=xs[:, rb : rb + nx, gsl, :], in_=psx[:]
                    )
                    ny = nx
                    ps = ppool.tile([128, ny * gh * V], mybir.dt.float32,
                                    tag="psB", bufs=3)
                    psy = ps[:, : ny * gh * V].rearrange(
                        "p (r g v) -> p r g v", r=ny, v=V
                    )
                    for ri in range(ny):
                        r = rb + ri
                        for q in range(NJ):
                            lhsT = fw_t[
                                :, (r * NQ + q) * 128 : (r * NQ + q + 1) * 128
                            ]
                            rhs = yin[:, 0:gh, q : q + 3 * (V - 1) + 1 : 3]
                            nc.tensor.matmul(
                                psy[:, ri, :, :], lhsT, rhs,
                                start=(ri == 0 and q == 0),
                                stop=(ri == ny - 1 and q == NJ - 1),
                            )
                    nc.scalar.copy(
                        out=ys[:, rb : rb + ny, gsl, :], in_=psy[:]
                    )
                    rb += nx

                # deferred inverse AFTER this chunk's forward matmuls so the
                # PE queue never stalls on the product engines
                while (
                    inv_emitted < len(_IGROUPS)
                    and _IGROUPS[inv_emitted][0] + _IGROUPS[inv_emitted][1] <= g0
                ):
                    emit_inverse(inv_emitted)
                    inv_emitted += 1

                # ---- pointwise products + v-sum trees (DVE + GpSimd) ----
                if ci >= len(_chunks) - DRAIN_CH:
                    pset = POOL_SET_D
                elif ci in MID_CH:
                    pset = POOL_SET_M
                else:
                    pset = POOL_SET

                def eng(c_, nm):
                    return nc.gpsimd if (c_, nm) in pset else nc.vector

                with nc.allow_low_precision("bf16 spectra products"):
                    # Pool muls FIRST (their consumers are emitted last so
                    # DVE never idles waiting on the slower engine).
                    p2a = wpool.tile([128, C * NJ, GHMAX, V], DT_MM, tag="p2a",
                                     bufs=2)
                    p2b = wpool.tile([128, C * NJ, GHMAX, V], DT_MM, tag="p2b",
                                     bufs=2)

                    def mul_p2(c, nm, dst):
                        xsl = (
                            xs[:, NJ:NR, gsl, c : c + V] if nm == "p2a"
                            else xs[:, 0:NJ, gsl, c : c + V]
                        )
                        ysl = (
                            ys[:, 0:NJ, gsl, :] if nm == "p2a"
                            else ys[:, NJ:NR, gsl, :]
                        )
                        eng(c, nm).tensor_mul(
                            dst[:, c * NJ : (c + 1) * NJ, :gh, :], xsl, ysl
                        )

                    for c in range(C):
                        for nm, dst in (("p2a", p2a), ("p2b", p2b)):
                            if (c, nm) in pset:
                                mul_p2(c, nm, dst)
                    # Zr path on DVE: p1 dim0 order (c, half, rh)
                    p1 = wpool.tile([128, C * NR, GHMAX, V], DT_MM,
                                    tag="p1", bufs=2)
                    for c in range(C):
                        nc.vector.tensor_mul(
                            p1[:, c * NR : (c + 1) * NR, :gh, :],
                            xs[:, :, gsl, c : c + V],
                            ys[:, :, gsl, :],
                        )
                    t4 = wpool.tile([128, C * NR, GHMAX, V // 2], DT_MM,
                                    tag="t4", bufs=2)
                    nc.vector.tensor_add(
                        t4[:, :, :gh, :], p1[:, :, :gh, 0 : V // 2],
                        p1[:, :, :gh, V // 2 : V],
                    )
                    t2 = wpool.tile([128, C * NR, GHMAX, V // 4], DT_MM,
                                    tag="t2", bufs=2)
                    nc.vector.tensor_add(
                        t2[:, :, :gh, :], t4[:, :, :gh, 0 : V // 4],
                        t4[:, :, :gh, V // 4 : V // 2],
                    )
                    ta = wpool.tile([128, C * NJ, GHMAX, 2], DT_MM, tag="ta",
                                    bufs=2)
                    for c in range(C):
                        nc.vector.tensor_add(
                            ta[:, c * NJ : (c + 1) * NJ, :gh, :],
                            t2[:, c * NR : c * NR + NJ, :gh, :],
                            t2[:, c * NR + NJ : (c + 1) * NR, :gh, :],
                        )
                    nc.vector.tensor_add(
                        zr[:, :, gsl, :].rearrange("p r g c -> p c r g"),
                        ta[:, :, :gh, 0].rearrange("p (c r) g -> p c r g", c=C),
                        ta[:, :, :gh, 1].rearrange("p (c r) g -> p c r g", c=C),
                    )
                    # remaining (DVE) cross muls, then the Zi tree
                    for c in range(C):
                        for nm, dst in (("p2a", p2a), ("p2b", p2b)):
                            if (c, nm) not in pset:
                                mul_p2(c, nm, dst)
                    dd = wpool.tile([128, C * NJ, GHMAX, V], DT_MM, tag="dd",
                                    bufs=2)
                    nc.vector.tensor_sub(
                        dd[:, :, :gh, :], p2a[:, :, :gh, :], p2b[:, :, :gh, :]
                    )
                    u4 = wpool.tile([128, C * NJ, GHMAX, V // 2], DT_MM,
                                    tag="u4", bufs=2)
                    nc.vector.tensor_add(
                        u4[:, :, :gh, :], dd[:, :, :gh, 0 : V // 2],
                        dd[:, :, :gh, V // 2 : V],
                    )
                    u2 = wpool.tile([128, C * NJ, GHMAX, V // 4], DT_MM,
                                    tag="u2", bufs=2)
                    nc.vector.tensor_add(
                        u2[:, :, :gh, :], u4[:, :, :gh, 0 : V // 4],
                        u4[:, :, :gh, V // 4 : V // 2],
                    )
                    nc.vector.tensor_add(
                        zi[:, :, gsl, :].rearrange("p r g c -> p c r g"),
                        u2[:, :, :gh, 0].rearrange("p (c r) g -> p c r g", c=C),
                        u2[:, :, :gh, 1].rearrange("p (c r) g -> p c r g", c=C),
                    )

            while inv_emitted < len(_IGROUPS):
                emit_inverse(inv_emitted)
                inv_emitted += 1

    nc.compile()
    return nc


def _prep_core_inputs(d1f, d2f, fw, mt, core):
    """d1f/d2f: [2048, 3000] fp32. Returns the in_map for `core`."""
    sl = slice(core * G, (core + 1) * G)
    x = d1f[sl]
    y = d2f[sl]
    xp = np.zeros((G, U * 128), dtype=np.float32)
    xp[:, XLEAD : XLEAD + NT] = x
    yp = np.zeros((G, W * 128), dtype=np.float32)
    yp[:, :NT] = y
    # device layouts: xpT[p, g, u] = xp[g, 128u + p]
    xpT = np.ascontiguousarray(xp.reshape(G, U, 128).transpose(2, 0, 1)).astype(NP_MM)
    ypT = np.ascontiguousarray(yp.reshape(G, W, 128).transpose(2, 0, 1)).astype(NP_MM)
    return {"xp": xpT, "yp": ypT, "fw": fw.astype(NP_MM), "mt": mt.astype(NP_MM)}


def kernel(data1: np.ndarray, data2: np.ndarray) -> np.ndarray:
    import time

    d1f = np.ascontiguousarray(data1, dtype=np.float32).reshape(-1, NT)
    d2f = np.ascontiguousarray(data2, dtype=np.float32).reshape(-1, NT)
    fw, mt = _const_tiles()

    t0 = time.time()
    if "nc" not in _PE_CACHE:
        _PE_CACHE["nc"] = build_kernel()
    nc = _PE_CACHE["nc"]
    print(f"[kernel] build+compile {time.time() - t0:.1f}s", file=sys.stderr,
          flush=True)

    in_maps = [_prep_core_inputs(d1f, d2f, fw, mt, i) for i in range(NCORES)]
    t0 = time.time()
    res = run_bass_kernel_spmd(nc, in_maps, core_ids=list(range(NCORES)))
    print(f"[kernel] spmd run {time.time() - t0:.1f}s", file=sys.stderr, flush=True)
    global LAST_EXEC_NS, LAST_TRACE
    LAST_EXEC_NS = res.exec_time_ns
    LAST_TRACE = res.instructions_and_trace
    if res.exec_time_ns is not None:
        print(f"[kernel] HW exec {res.exec_time_ns} ns", file=sys.stderr, flush=True)

    # slot order: (jg0,c0) (jg0,c1) (jg1,c0) (jg1,c1) (jg2,c0)
    slot_of = {(0, 0): 0, (0, 1): 1, (1, 0): 2, (1, 1): 3, (2, 0): 4}
    outs = []
    for i in range(NCORES):
        o = res.results[i]["out"]  # [128, G, 5]
        full = np.empty((G, LAGS), dtype=np.float32)
        for jg in range(NJ):
            for c in range(C):
                if (jg, c) not in slot_of:
                    continue
                lo = 384 * c + 128 * jg
                if lo >= LAGS:
                    continue
                n = min(128, LAGS - lo)
                full[:, lo : lo + n] = o[:n, :, slot_of[(jg, c)]].T
        outs.append(full)
    return np.concatenate(outs, axis=0).reshape(NB_PAIRS, NCH, LAGS)


# revision 12
# speedup vs baseline: 1.2133x; 1.0428x over previous
"""Trainium2 Bass kernel: batched time-domain cross-correlation.

Computes, for each of 2048 (=64x32) independent pairs (fp32):
    out[g, l] = sum_k d1[g, k + l - 301] * d2[g, k],   l in [0, 603)

Algorithm: overlap-save block correlation in a half-shift (negacyclic)
real-DFT basis; every matmul has a *shared* stationary operand (the
transform matrices) and batches all pairs in the moving operand:

  xp = d1 zero-padded/shifted; y = d2 zero-padded.
  out[B*c + j] = sum_v corr(w_{v+c}, y_v)[j]     (j in [0, B))
    w_s = xp-window at stride B, length N=2B
    y_v = y[B*v : B*v + B]    (blocks, zero-padded to N)
  Per-block circular corr via length-N negacyclic real DFT:
    bins k: Ur[k] = sum_n u[n] cos(pi n (2k+1)/N)
            Ui[k] = -sum_n u[n] sin(pi n (2k+1)/N),  k in [0, B)
    Z = X * conj(Y):  Zr = XrYr + XiYi ; Zi = XiYr - XrYi
    z[0:B] = Minv @ [Zr; Zi]  (exact: aliasing only corrupts j > B)

vs the first version of this kernel:
  - xp keeps only 45 leading / 27 trailing zeros (24 chunks, not 30);
    window chunks falling in the implicit zero region are skipped
    (x-forward matmul rows 82944 -> 72192 per core).
  - Zi is a single plane (d = XiYr - XrYi subtracted on DVE), so the
    inverse has 2 stationary planes, not 3 (rows 13824 -> 7680), and
    (c=1, jg=2) lag outputs >= 640 are never computed (lags end at 602).
  - product/tree ops are fused across all 6 bin groups per op (fewer,
    fatter DVE instructions; the fixed SBUF-access cost per op is large).
  - a tunable subset of the product muls runs on GpSimd.

Sharding: data-parallel over the 2048 pairs, 256 pairs per core, 8 cores.
"""

import math
import os
import sys

import ml_dtypes
import numpy as np

if "/opt/trn_rl_repo" not in sys.path:  # harness safety; axon site usually set
    sys.path.insert(0, "/opt/trn_rl_repo")

import concourse.bacc as bacc
import concourse.bass as bass
import concourse.mybir as mybir
import concourse.tile as tile
from concourse.bass_utils import run_bass_kernel_spmd

# ---- problem constants (hardcoded per contest contract) ----
NB_PAIRS, NCH, NT = 64, 32, 3000
LAGS = 603
SHIFT = 301  # NLAG + 1
NCORES = 8
G = (NB_PAIRS * NCH) // NCORES  # 256 pairs per core

# ---- algorithm constants ----
B = 384  # lag/block granularity; N = 2B
N = 2 * B
V = 8  # y blocks (ceil 3000/384)
C = 2  # output lag blocks (ceil 603/384)
S = V + C - 1  # x windows
BS = B // 128  # 3
NQ = N // 128  # 6 contraction chunks of a full window
NJ = B // 128  # 3
NR = 2 * NJ  # 6 bin groups (Re 0..2, Im 3..5)
XLEAD = 45  # leading zeros kept in xp (301 = 2*128 + 45)
U = 24  # xp chunks: 45 + 3000 + 27 = 3072
W = 24  # y chunks: 3000 + 72 = 3072

# x-forward matmul emission: (q, s0, s1, start, stop) per (r, chunk).
# Window s uses xp chunk u = 3s + q - 2; chunks u<0 / u>=24 are implicit
# zeros (skipped).  start/stop flags per psum column range:
#   w0 first=q2 last=q5; w1-6 first=q0 last=q5; w7 first=q0 last=q4;
#   w8 first=q0 last=q1.
XMM = [
    (0, 1, 9, True, False),
    (1, 1, 9, False, False),
    (2, 0, 8, False, False),
    (3, 0, 8, False, False),
    (4, 0, 8, False, False),
    (5, 0, 7, False, True),
]

# inverse output groups: (jg, cp) — lag l = 384c + 128jg + p; lags < 603
# need (c=0, jg=0..2) and (c=1, jg=0..1).
INVJG = [(0, 2), (1, 2), (2, 1)]

DT_MM = mybir.dt.bfloat16
NP_MM = ml_dtypes.bfloat16

# pairs per chunk: psum for x-fwd is [gh, S] fp32 <= 512 -> gh <= 56.
# Small head chunk (overlap DMA latency + PE ramp), geometric tail so the
# product drain after the last forwards is short.
_CH = [int(t) for t in os.environ.get(
    "KCHUNKS", "32,56,56,56,28,16,8,4").split(",")]
assert sum(_CH) == G
_chunks = []
_g = 0
for _w in _CH:
    _chunks.append((_g, _w))
    _g += _w
GHMAX = max(gh for _, gh in _chunks)
# inverse groups == chunks (inverse for chunk i is emitted during chunk i+1)
_IGROUPS = list(_chunks)

# which product muls go to GpSimd: list of (c, op) with op in
# {"p1", "p2a", "p2b"}; tuned so Pool busy ~= DVE busy ~= PE busy.
# KPOOL applies to steady-state chunks, KPOOLD to the drain chunks
# (the last DRAIN_CH chunks, where products must finish fast).
def _parse_pool(s):
    out = set()
    if s:
        for tok in s.split(","):
            c_, nm = tok.split(":")
            out.add((int(c_), nm))
    return out

POOL_SET = _parse_pool(os.environ.get("KPOOL", "0:p2a,1:p2a,1:p2b"))
POOL_SET_M = _parse_pool(
    os.environ.get("KPOOLM", "0:p2a,1:p2a,1:p2b"))
POOL_SET_D = _parse_pool(os.environ.get("KPOOLD", "0:p2a,1:p2b"))
DRAIN_CH = int(os.environ.get("KDRAIN", "3"))
MID_CH = set(
    int(t) for t in os.environ.get("KMID", "1,2,3").split(",") if t)

_PE_CACHE = {}
LAST_EXEC_NS = None
LAST_TRACE = None


def _matrices():
    n = np.arange(N, dtype=np.float64)[:, None]
    k = np.arange(B, dtype=np.float64)[None, :]
    theta = np.pi * n * (2 * k + 1) / N
    ffull = np.concatenate([np.cos(theta), -np.sin(theta)], axis=1)  # [N, 2B]
    minv = np.linalg.inv(ffull.T)[:B, :]  # [B, 2B]
    return ffull.astype(np.float32), minv.astype(np.float32)


def _const_tiles():
    """FW [128, NR*NQ*128]: FW[i, ((r*NQ)+q)*128 + col] = Ffull[128q+i, 128r+col]
    MT [128, 2*NJ*NJ*128]: for pl in {Mr, Mi}:
        MT[i, ((pl*NJ + rh)*NJ + jg)*128 + col] = M[128jg + col, 128rh + i]
    """
    ffull, minv = _matrices()
    fw = np.zeros((128, NR * NQ * 128), dtype=np.float32)
    for q in range(NQ):
        for r in range(NR):
            fw[:, (r * NQ + q) * 128 : (r * NQ + q + 1) * 128] = ffull[
                128 * q : 128 * (q + 1), 128 * r : 128 * (r + 1)
            ]
    mr = minv[:, :B]
    mi = minv[:, B:]
    mats = [mr, mi]
    mt = np.zeros((128, 2 * NJ * NJ * 128), dtype=np.float32)
    for pl in range(2):
        for rh in range(NJ):
            for jg in range(NJ):
                blk = mats[pl][128 * jg : 128 * (jg + 1), 128 * rh : 128 * (rh + 1)]
                base = ((pl * NJ + rh) * NJ + jg) * 128
                mt[:, base : base + 128] = blk.T
    return fw, mt


def build_kernel():
    nc = bacc.Bacc(
        "TRN2",
        target_bir_lowering=False,
        debug=False,
        num_devices=NCORES,
    )

    xp_d = nc.dram_tensor("xp", [128, G, U], DT_MM, kind="ExternalInput")
    yp_d = nc.dram_tensor("yp", [128, G, W], DT_MM, kind="ExternalInput")
    fw_d = nc.dram_tensor("fw", [128, NR * NQ * 128], DT_MM, kind="ExternalInput")
    mt_d = nc.dram_tensor("mt", [128, 2 * NJ * NJ * 128], DT_MM, kind="ExternalInput")
    out_d = nc.dram_tensor("out", [128, G, 5], mybir.dt.float32,
                           kind="ExternalOutput")

    with tile.TileContext(nc, trace_sim=False) as tc:
        with (
            tc.tile_pool(name="const", bufs=1) as cpool,
            tc.tile_pool(name="io", bufs=2) as iopool,
            tc.tile_pool(name="spec", bufs=2) as spool,
            tc.tile_pool(name="work", bufs=2) as wpool,
            tc.tile_pool(name="zpool", bufs=1) as zpool,
            tc.tile_pool(name="psum", bufs=1, space=bass.MemorySpace.PSUM) as ppool,
        ):
            fw_t = cpool.tile([128, NR * NQ * 128], DT_MM, tag="fw")
            mt_t = cpool.tile([128, 2 * NJ * NJ * 128], DT_MM, tag="mt")
            # spectra for all pairs: [128 bins-in-group, r-group, pair, slot]
            xs = cpool.tile([128, NR, G, S], DT_MM, tag="xs")
            ys = cpool.tile([128, NR, G, V], DT_MM, tag="ys")
            # accumulated spectral products, c-major so last free dim packs
            zr = zpool.tile([128, NJ, G, C], DT_MM, tag="zr")
            zi = zpool.tile([128, NJ, G, C], DT_MM, tag="zi")

            outt = iopool.tile([128, G, 5], mybir.dt.float32, tag="outt",
                               bufs=1)

            def emit_inverse(fgi):
                ig0, ign = _IGROUPS[fgi]
                gsl = slice(ig0, ig0 + ign)
                ps = ppool.tile([128, GHMAX, 5], mybir.dt.float32,
                                tag="psC", bufs=2)
                first = True
                slot = 0
                for jg, cp in INVJG:
                    psv = ps[:, :ign, slot : slot + cp]
                    nmm = 2 * NJ
                    i = 0
                    for pl, zt in ((0, zr), (1, zi)):
                        for rh in range(NJ):
                            # moving [ign, cp]: pair-major, c inner
                            rhs = zt[:, rh, gsl, 0:cp]
                            lhsT = mt_t[
                                :,
                                ((pl * NJ + rh) * NJ + jg) * 128 :
                                ((pl * NJ + rh) * NJ + jg + 1) * 128,
                            ]
                            nc.tensor.matmul(
                                psv, lhsT, rhs,
                                start=first, stop=(i == nmm - 1),
                            )
                            first = False
                            i += 1
                    slot += cp
                nc.scalar.copy(out=outt[:, gsl, :], in_=ps[:, :ign, :])
                nc.sync.dma_start(
                    out_d.ap()[:, gsl, :], outt[:, gsl, :]
                )

            inv_emitted = 0
            zi_dep = None
            for ci, (g0, gh) in enumerate(_chunks):
                gsl = slice(g0, g0 + gh)
                xin = iopool.tile([128, GHMAX, U], DT_MM, tag="xin", bufs=3)
                yin = iopool.tile([128, GHMAX, W], DT_MM, tag="yin", bufs=3)
                if ci == 0:
                    nc.sync.dma_start(
                        fw_t[:, 0 : NQ * 128], fw_d.ap()[:, 0 : NQ * 128]
                    )
                nc.sync.dma_start(xin[:, :gh, :], xp_d.ap()[:, gsl, :])
                nc.sync.dma_start(yin[:, :gh, :], yp_d.ap()[:, gsl, :])
                if ci == 0:
                    for r in range(1, NR):
                        nc.sync.dma_start(
                            fw_t[:, r * NQ * 128 : (r + 1) * NQ * 128],
                            fw_d.ap()[:, r * NQ * 128 : (r + 1) * NQ * 128],
                        )
                    nc.sync.dma_start(mt_t[:], mt_d.ap())

                # ---- forward transforms, x and y interleaved per bin
                # group; small chunks pack several r-groups per psum bank so
                # one Act copy drains several (the per-op init cost is big)
                rpb = NR
                while rpb * gh * S > 512:
                    rpb -= 1
                rpb = max(1, min(rpb, 3))
                rb = 0
                while rb < NR:
                    nx = min(rpb, NR - rb)
                    ps = ppool.tile([128, nx * gh * S], mybir.dt.float32,
                                    tag="psA", bufs=3)
                    psx = ps[:, : nx * gh * S].rearrange(
                        "p (r g s) -> p r g s", r=nx, s=S
                    )
                    for ri in range(nx):
                        r = rb + ri
                        first = ri == 0
                        for q, s0, s1, st, sp in XMM:
                            lhsT = fw_t[
                                :, (r * NQ + q) * 128 : (r * NQ + q + 1) * 128
                            ]
                            u0 = 3 * s0 + q - 2
                            u1 = 3 * (s1 - 1) + q - 2
                            rhs = xin[:, 0:gh, u0 : u1 + 1 : 3]
                            nc.tensor.matmul(
                                psx[:, ri, :, s0:s1], lhsT, rhs,
                                start=(first and st),
                                stop=(sp and ri == nx - 1),
                            )
                    nc.scalar.copy(
                        out=xs[:, rb : rb + nx, gsl, :], in_=psx[:]
                    )
                    ny = nx
                    ps = ppool.tile([128, ny * gh * V], mybir.dt.float32,
                                    tag="psB", bufs=3)
                    psy = ps[:, : ny * gh * V].rearrange(
                        "p (r g v) -> p r g v", r=ny, v=V
                    )
                    for ri in range(ny):
                        r = rb + ri
                        for q in range(NJ):
                            lhsT = fw_t[
                                :, (r * NQ + q) * 128 : (r * NQ + q + 1) * 128
                            ]
                            rhs = yin[:, 0:gh, q : q + 3 * (V - 1) + 1 : 3]
                            nc.tensor.matmul(
                                psy[:, ri, :, :], lhsT, rhs,
                                start=(ri == 0 and q == 0),
                                stop=(ri == ny - 1 and q == NJ - 1),
                            )
                    nc.scalar.copy(
                        out=ys[:, rb : rb + ny, gsl, :], in_=psy[:]
                    )
                    rb += nx

                # deferred inverse AFTER this chunk's forward matmuls so the
                # PE queue never stalls on the product engines
                while inv_emitted <= ci - 2:
                    emit_inverse(inv_emitted)
                    inv_emitted += 1

                # ---- pointwise products + v-sum trees (DVE + GpSimd) ----
                if ci >= len(_chunks) - DRAIN_CH:
                    pset = POOL_SET_D
                elif ci in MID_CH:
                    pset = POOL_SET_M
                else:
                    pset = POOL_SET

                def eng(c_, nm):
                    return nc.gpsimd if (c_, nm) in pset else nc.vector

                def emit_zi_tree(dep):
                    dgsl, dgh, dp2a, dp2b = dep
                    with nc.allow_low_precision("bf16 spectra products"):
                        dd = wpool.tile([128, C * NJ, GHMAX, V], DT_MM,
                                        tag="dd", bufs=2)
                        nc.vector.tensor_sub(
                            dd[:, :, :dgh, :], dp2a[:, :, :dgh, :],
                            dp2b[:, :, :dgh, :],
                        )
                        u4 = wpool.tile([128, C * NJ, GHMAX, V // 2], DT_MM,
                                        tag="u4", bufs=2)
                        nc.vector.tensor_add(
                            u4[:, :, :dgh, :], dd[:, :, :dgh, 0 : V // 2],
                            dd[:, :, :dgh, V // 2 : V],
                        )
                        u2 = wpool.tile([128, C * NJ, GHMAX, V // 4], DT_MM,
                                        tag="u2", bufs=2)
                        nc.vector.tensor_add(
                            u2[:, :, :dgh, :], u4[:, :, :dgh, 0 : V // 4],
                            u4[:, :, :dgh, V // 4 : V // 2],
                        )
                        nc.vector.tensor_add(
                            zi[:, :, dgsl, :].rearrange("p r g c -> p c r g"),
                            u2[:, :, :dgh, 0].rearrange(
                                "p (c r) g -> p c r g", c=C),
                            u2[:, :, :dgh, 1].rearrange(
                                "p (c r) g -> p c r g", c=C),
                        )

                with nc.allow_low_precision("bf16 spectra products"):
                    # Pool muls FIRST; their consumer (the Zi tree) is
                    # deferred a full chunk so DVE never waits on Pool.
                    p2a = wpool.tile([128, C * NJ, GHMAX, V], DT_MM, tag="p2a",
                                     bufs=2)
                    p2b = wpool.tile([128, C * NJ, GHMAX, V], DT_MM, tag="p2b",
                                     bufs=2)

                    def mul_p2(c, nm, dst):
                        xsl = (
                            xs[:, NJ:NR, gsl, c : c + V] if nm == "p2a"
                            else xs[:, 0:NJ, gsl, c : c + V]
                        )
                        ysl = (
                            ys[:, 0:NJ, gsl, :] if nm == "p2a"
                            else ys[:, NJ:NR, gsl, :]
                        )
                        eng(c, nm).tensor_mul(
                            dst[:, c * NJ : (c + 1) * NJ, :gh, :], xsl, ysl
                        )

                    for c in range(C):
                        for nm, dst in (("p2a", p2a), ("p2b", p2b)):
                            if (c, nm) in pset:
                                mul_p2(c, nm, dst)
                    # Zr path on DVE: p1 dim0 order (c, half, rh)
                    p1 = wpool.tile([128, C * NR, GHMAX, V], DT_MM,
                                    tag="p1", bufs=2)
                    for c in range(C):
                        nc.vector.tensor_mul(
                            p1[:, c * NR : (c + 1) * NR, :gh, :],
                            xs[:, :, gsl, c : c + V],
                            ys[:, :, gsl, :],
                        )
                    t4 = wpool.tile([128, C * NR, GHMAX, V // 2], DT_MM,
                                    tag="t4", bufs=2)
                    nc.vector.tensor_add(
                        t4[:, :, :gh, :], p1[:, :, :gh, 0 : V // 2],
                        p1[:, :, :gh, V // 2 : V],
                    )
                    t2 = wpool.tile([128, C * NR, GHMAX, V // 4], DT_MM,
                                    tag="t2", bufs=2)
                    nc.vector.tensor_add(
                        t2[:, :, :gh, :], t4[:, :, :gh, 0 : V // 4],
                        t4[:, :, :gh, V // 4 : V // 2],
                    )
                    ta = wpool.tile([128, C * NJ, GHMAX, 2], DT_MM, tag="ta",
                                    bufs=2)
                    for c in range(C):
                        nc.vector.tensor_add(
                            ta[:, c * NJ : (c + 1) * NJ, :gh, :],
                            t2[:, c * NR : c * NR + NJ, :gh, :],
                            t2[:, c * NR + NJ : (c + 1) * NR, :gh, :],
                        )
                    nc.vector.tensor_add(
                        zr[:, :, gsl, :].rearrange("p r g c -> p c r g"),
                        ta[:, :, :gh, 0].rearrange("p (c r) g -> p c r g", c=C),
                        ta[:, :, :gh, 1].rearrange("p (c r) g -> p c r g", c=C),
                    )
                    # remaining (DVE) cross muls
                    for c in range(C):
                        for nm, dst in (("p2a", p2a), ("p2b", p2b)):
                            if (c, nm) not in pset:
                                mul_p2(c, nm, dst)

                # Zi tree for the PREVIOUS chunk (pool muls have finished)
                if zi_dep is not None:
                    emit_zi_tree(zi_dep)
                zi_dep = (gsl, gh, p2a, p2b)

            # drain: inverses for ready groups, last Zi tree, last inverses
            while inv_emitted < len(_IGROUPS) - 1:
                emit_inverse(inv_emitted)
                inv_emitted += 1
            if zi_dep is not None:
                emit_zi_tree(zi_dep)
            emit_inverse(inv_emitted)

    nc.compile()
    return nc


def _prep_core_inputs(d1f, d2f, fw, mt, core):
    """d1f/d2f: [2048, 3000] fp32. Returns the in_map for `core`."""
    sl = slice(core * G, (core + 1) * G)
    x = d1f[sl]
    y = d2f[sl]
    xp = np.zeros((G, U * 128), dtype=np.float32)
    xp[:, XLEAD : XLEAD + NT] = x
    yp = np.zeros((G, W * 128), dtype=np.float32)
    yp[:, :NT] = y
    # device layouts: xpT[p, g, u] = xp[g, 128u + p]
    xpT = np.ascontiguousarray(xp.reshape(G, U, 128).transpose(2, 0, 1)).astype(NP_MM)
    ypT = np.ascontiguousarray(yp.reshape(G, W, 128).transpose(2, 0, 1)).astype(NP_MM)
    return {"xp": xpT, "yp": ypT, "fw": fw.astype(NP_MM), "mt": mt.astype(NP_MM)}


def kernel(data1: np.ndarray, data2: np.ndarray) -> np.ndarray:
    import time

    d1f = np.ascontiguousarray(data1, dtype=np.float32).reshape(-1, NT)
    d2f = np.ascontiguousarray(data2, dtype=np.float32).reshape(-1, NT)
    fw, mt = _const_tiles()

    t0 = time.time()
    if "nc" not in _PE_CACHE:
        _PE_CACHE["nc"] = build_kernel()
    nc = _PE_CACHE["nc"]
    print(f"[kernel] build+compile {time.time() - t0:.1f}s", file=sys.stderr,
          flush=True)

    in_maps = [_prep_core_inputs(d1f, d2f, fw, mt, i) for i in range(NCORES)]
    t0 = time.time()
    res = run_bass_kernel_spmd(nc, in_maps, core_ids=list(range(NCORES)))
    print(f"[kernel] spmd run {time.time() - t0:.1f}s", file=sys.stderr, flush=True)
    global LAST_EXEC_NS, LAST_TRACE
    LAST_EXEC_NS = res.exec_time_ns
    LAST_TRACE = res.instructions_and_trace
    if res.exec_time_ns is not None:
        print(f"[kernel] HW exec {res.exec_time_ns} ns", file=sys.stderr, flush=True)

    # slot order: (jg0,c0) (jg0,c1) (jg1,c0) (jg1,c1) (jg2,c0)
    slot_of = {(0, 0): 0, (0, 1): 1, (1, 0): 2, (1, 1): 3, (2, 0): 4}
    outs = []
    for i in range(NCORES):
        o = res.results[i]["out"]  # [128, G, 5]
        full = np.empty((G, LAGS), dtype=np.float32)
        for jg in range(NJ):
            for c in range(C):
                if (jg, c) not in slot_of:
                    continue
                lo = 384 * c + 128 * jg
                if lo >= LAGS:
                    continue
                n = min(128, LAGS - lo)
                full[:, lo : lo + n] = o[:n, :, slot_of[(jg, c)]].T
        outs.append(full)
    return np.concatenate(outs, axis=0).reshape(NB_PAIRS, NCH, LAGS)
